# revision 29
# baseline (speedup 1.0000x reference)
"""Low-rank layer y = x @ (U diag(s) V^T)^T on 8 TRN2 NeuronCores.

Factored as two thin matmuls per core (data-parallel over batch, 1 batch/core):
  stage 1: t[r, n]  = sum_i (V*s)[i, r] * x[n, i]   (contraction i on partitions)
  stage 2: y[n, o]  = sum_r t[r, n] * U[o, r]       (contraction r on partitions)

All matmul operands bf16 (PSUM accumulates fp32). The PE floor is ~110.6us
(512 matmuls x 216ns at 2.4GHz); everything else is overhead management:
(a) x chunk 0 loads in matmul-tile pieces at the head of the Sync HWDGE ring
    so the first matmul fires ~10.5us in (issue ~0.7us + completion receipt
    ~2.3us after the ~7.3us engine-preamble barrier are irreducible);
(b) early vs weight groups are per-chunk and ride the Activation HWDGE ring
    (Scalar engine) so Sync's FIFO stays pure-x through the ramp; later
    groups interleave into the Sync ring between x chunks (FIFO = JIT pacing);
(c) stage 2 runs ot-pair-major (finish each psum tile h0+h1 before the next)
    with PSUM->SBUF copies alternating Vector/Scalar and quarter y stores
    chasing the copies, so the y-write pipeline trails the PE by only ~1
    quarter and the final store's ~2us completion receipt starts right after
    the last copy;
(d) the TileContext exit skips semaphore clears + both exit barriers (the
    engine preambles re-init semaphores each NEFF execution).

DO NOT wake the PE (or GPSIMD's SWDGE) before ~10us: the clock governor
punishes early compute-engine activity with a sustained lower PE clock for
the whole run -- measured 2.2GHz (236ns/matmul) for PE busy from 8.4us,
2.0GHz (259ns) for an early busy->idle->busy pattern, and 2.0GHz for
pre-barrier GPSIMD SWDGE prefetch, vs 2.4GHz (216ns) when quiet until
~10.4us. Warm-up dummies and pre-barrier prefetch are net losers here.

Host prep: fold s into V, transpose x per core so the contraction dim lands on
SBUF partitions, pre-tile weights into partition-major layouts.
"""

import numpy as np
import ml_dtypes

import concourse.bass as bass
import concourse.mybir as mybir
import concourse.tile as tile
from concourse.tile import ScopedClock
from concourse.bass_utils import run_bass_kernel_spmd

P = 128
B = 8
TOKENS = 2048
D_IN = 4096
D_OUT = 4096
R = 256
I_CHUNKS = D_IN // P  # 32
R_HALVES = R // P  # 2
N_TILE = 512
T_TILES = TOKENS // N_TILE  # 4
TOK_BLKS = TOKENS // P  # 16
O_TILES = D_OUT // N_TILE  # 8


def _patched_drain_and_barrier(self, tick_clock, wait_clock):
    # This walrus build's CoreV3 CTRL lowering accepts at most one sync-wait
    # on the TileContext-exit SP Drain; split the global-clock waits across a
    # chain of SP nops (one wait each) emitted just before the drain.
    # Also: skip clear_and_free_semaphores + the second all_engine_barrier
    # (the engine preambles re-init semaphores each NEFF execution; the exit
    # clears cost ~1us of teardown).
    nc = self.nc
    lead = nc.sync.nop(nofuse=True, hint="tile_drain_wait_split")
    wait_clock.add_sem_waits(lead.ins, ScopedClock({None: tick_clock.global_clock}))
    si = lead.ins.sync_info
    waits = list(si.on_wait or [])
    if len(waits) > 1:
        si.on_wait = waits[:1]
        for w in waits[1:]:
            extra = nc.sync.nop(nofuse=True, hint="tile_drain_wait_split")
            esi = extra.ins.sync_info
            if esi is None:
                extra.ins.sync_info = mybir.SyncInfo(on_wait=[w], on_update=[])
            else:
                esi.on_wait = [w]
    nc.sync.drain()
    assert self.sems is not None
    popped = nc._tile_sem_poison_stack.pop()
    assert popped is self._sem_poison


def _install_drain_patch():
    if not getattr(tile.TileContext, "_drain_patch_installed", False):
        tile.TileContext._drain_and_barrier = _patched_drain_and_barrier
        tile.TileContext._drain_patch_installed = True


def _legalize_waits(nc):
    # This walrus build accepts at most one sync-wait per instruction.
    # Hoist extra waits onto same-engine nops inserted just before the
    # offending instruction (same engine queue -> identical blocking).
    for fn in nc.m.functions:
        for bb in fn.blocks:
            new_list = []
            for inst in list(bb.instructions):
                si = inst.sync_info
                waits = list(si.on_wait) if si and si.on_wait else []
                if len(waits) > 1:
                    for w in waits[:-1]:
                        nop = nc.engines[inst.engine].nop(
                            nofuse=True, hint="wait_split"
                        )
                        cur = nc.cur_bb.bb.instructions
                        assert cur[-1] is nop.ins
                        cur.pop()
                        nsi = nop.ins.sync_info
                        if nsi is None:
                            nop.ins.sync_info = mybir.SyncInfo(
                                on_wait=[w], on_update=[]
                            )
                        else:
                            nsi.on_wait = [w]
                        new_list.append(nop.ins)
                    si.on_wait = [waits[-1]]
                new_list.append(inst)
            bb.instructions[:] = new_list


# vs weight-load groups: per-chunk for the first 4 (so chunk c never waits on
# a bulk group during the DMA ramp), then bulk groups streamed just-in-time.
# Group 0 leads the Sync ring (first matmul dep); groups 1-4 go on the
# Activation HWDGE ring (Scalar engine, idle in stage 1) so Sync's FIFO
# stays pure-x early; groups 5-7 interleave into the Sync ring between x
# chunk issues for just-in-time pacing.
VS_SIZES = [1, 1, 1, 1, 4, 8, 8, 8]
VS_STARTS = [0, 1, 2, 3, 4, 8, 16, 24]
VS_GROUPS = len(VS_SIZES)
VS_SYNC_AT = {6: 5, 13: 6, 21: 7}  # x-chunk position -> group issued on Sync
# first x chunk arrives in halves: the first semaphore covers tt0+tt1
# (~1.7us of cold-clock work) while the second half lands. Finer splits
# serialize more per-DMA completion receipts (~2.3us each) and starve the
# chunk-1..3 window instead.
X0_EDGES = [0, 1024, 2048]


def _build(iodt=mybir.dt.bfloat16):
    f32 = mybir.dt.float32
    nc = bass.Bass()
    xT_d = nc.declare_dram_parameter("xT", [D_IN, TOKENS], iodt, isOutput=False)
    vs_d = [
        nc.declare_dram_parameter(f"vs{g}", [P, VS_SIZES[g], R], iodt, isOutput=False)
        for g in range(VS_GROUPS)
    ]
    ut_d = nc.declare_dram_parameter("ut", [R_HALVES, P, D_OUT], iodt, isOutput=False)
    y_d = nc.declare_dram_parameter("y", [TOKENS, D_OUT], iodt, isOutput=True)

    with tile.TileContext(nc) as tc:
        with (
            tc.tile_pool(name="consts", bufs=1) as consts,
            tc.tile_pool(name="xp", bufs=12) as xp,
            tc.tile_pool(name="t2p", bufs=1) as t2p,
            tc.tile_pool(name="yp", bufs=5) as yp,
            tc.tile_pool(name="psum", bufs=8, space="PSUM") as psum,
        ):
            # PSUM->SBUF copies round-robin over both PSUM-capable engines so
            # no single engine's copy chain gates PSUM-bank reuse by the
            # matmuls (GPSIMD cannot read PSUM).
            copy_engines = [nc.vector.tensor_copy, nc.scalar.copy]

            vs_sb = [
                consts.tile([P, VS_SIZES[g], R], iodt, name=f"vs{g}")
                for g in range(VS_GROUPS)
            ]
            ut_sb = [
                consts.tile([P, D_OUT], iodt, name=f"ut{h}") for h in range(R_HALVES)
            ]
            # stage-1 PSUM accumulators: t[r, n] over the 32 i-chunks
            psum_t = [
                [psum.tile([P, N_TILE], f32, tag="ps", name="ps_t") for _ in range(T_TILES)]
                for _ in range(R_HALVES)
            ]

            # NOTE: do NOT add PE warm-up dummies. The PE clock governor
            # punishes early activity: PE busy from ~8.4us locks a sustained
            # 2.2GHz (236ns/matmul), from ~8.7us-with-idle-gap locks 2.0GHz
            # (259ns); quiet until ~10.4us gives full 2.4GHz (216ns).

            # Act ring (Scalar) head: ALL early vs groups, vs0 first -- its
            # 64KB lands in ~3us there while the Sync ring is pure-x from
            # its very first issue, so the x pieces' issue+receipt chain
            # starts ~0.7us earlier and nothing shares their critical path.
            for g in (0, 1, 2, 3, 4):
                nc.scalar.dma_start(out=vs_sb[g][:], in_=vs_d[g][:])
            xt0 = xp.tile([P, TOKENS], iodt, tag="xt", name="xt")
            for e0, e1 in zip(X0_EDGES, X0_EDGES[1:]):
                nc.sync.dma_start(out=xt0[:, e0:e1], in_=xT_d[0:P, e0:e1])

            # stage 1: accumulate t[r, n] over the 32 i-chunks
            for c in range(I_CHUNKS):
                if c == 0:
                    xt = xt0
                else:
                    xt = xp.tile([P, TOKENS], iodt, tag="xt", name="xt")
                    nc.sync.dma_start(out=xt[:], in_=xT_d[c * P : (c + 1) * P, :])
                if c in VS_SYNC_AT:
                    g = VS_SYNC_AT[c]
                    nc.sync.dma_start(out=vs_sb[g][:], in_=vs_d[g][:])
                g = max(i for i in range(VS_GROUPS) if VS_STARTS[i] <= c)
                # chunks 0-1 run tt-major so the matmul order tracks the
                # arrival order of the x pieces (h-major would stall tt=2/3
                # on the trailing piece twice)
                s1_order = (
                    [(h, tt) for tt in range(T_TILES) for h in range(R_HALVES)]
                    if c < 2
                    else [(h, tt) for h in range(R_HALVES) for tt in range(T_TILES)]
                )
                for h, tt in s1_order:
                    nc.tensor.matmul(
                        psum_t[h][tt],
                        vs_sb[g][:, c - VS_STARTS[g], h * P : (h + 1) * P],
                        xt[:, tt * N_TILE : (tt + 1) * N_TILE],
                        start=(c == 0),
                        stop=(c == I_CHUNKS - 1),
                    )

            # ut rides behind the whole x stream: it lands in the DMA idle gap
            # at the stage-1->stage-2 transition instead of inflating stage
            # 1's DMA-bound window. Quarter DMAs in ot order so stage 2's
            # first matmuls unblock as soon as the leading quarters arrive.
            uq = D_OUT // 4
            for qi in range(4):
                for h in range(R_HALVES):
                    nc.sync.dma_start(
                        out=ut_sb[h][:, qi * uq : (qi + 1) * uq],
                        in_=ut_d[h, :, qi * uq : (qi + 1) * uq],
                    )

            # t back to SBUF (stage-2 stationary operand must live in SBUF);
            # tt-major order so stage 2's first token blocks unblock first.
            t2_sb = t2p.tile([P, R_HALVES, TOKENS], iodt)
            for k, (tt, h) in enumerate(
                (tt, h) for tt in range(T_TILES) for h in range(R_HALVES)
            ):
                copy_engines[k % 2](
                    out=t2_sb[:, h, tt * N_TILE : (tt + 1) * N_TILE],
                    in_=psum_t[h][tt],
                )

            # stage 2: y[n, o] accumulated over the 2 r-halves
            for tb in range(TOK_BLKS):
                y_sb = yp.tile([P, D_OUT], iodt, tag="yt", name="yt")
                psum_y = [
                    psum.tile([P, N_TILE], f32, tag="ps", name="ps_y")
                    for _ in range(O_TILES)
                ]
                # ot-pair-major: complete each psum tile (h0 then h1) before
                # the next ot, so copies chase ~1.7us behind the matmuls
                # instead of backlogging behind the h1 pass, and quarter
                # stores chase the copies -- this keeps the y-write pipeline
                # only ~1 quarter behind the PE, so the final store (and its
                # ~2us completion receipt) starts right after the last copy.
                last = tb == TOK_BLKS - 1
                for ot in range(O_TILES):
                    for h in range(R_HALVES):
                        nc.tensor.matmul(
                            psum_y[ot],
                            t2_sb[:, h, tb * P : (tb + 1) * P],
                            ut_sb[h][:, ot * N_TILE : (ot + 1) * N_TILE],
                            start=(h == 0),
                            stop=(h == R_HALVES - 1),
                        )
                    if last and ot == O_TILES - 1:
                        # the kernel's very last o-tile: copy in halves on
                        # BOTH engines and store it alone (128KB), so the
                        # final DMA (whose ~1.5us completion receipt gates
                        # the exit drain) starts ~0.6us after the last matmul
                        hn = N_TILE // 2
                        for k in range(2):
                            copy_engines[k](
                                out=y_sb[:, ot * N_TILE + k * hn : ot * N_TILE + (k + 1) * hn],
                                in_=psum_y[ot][:, k * hn : (k + 1) * hn],
                            )
                        nc.sync.dma_start(
                            out=y_d[tb * P : (tb + 1) * P, ot * N_TILE : D_OUT],
                            in_=y_sb[:, ot * N_TILE : D_OUT],
                        )
                        continue
                    copy_engines[ot % 2](
                        out=y_sb[:, ot * N_TILE : (ot + 1) * N_TILE],
                        in_=psum_y[ot],
                    )
                    if last and ot == O_TILES - 2:
                        nc.sync.dma_start(
                            out=y_d[tb * P : (tb + 1) * P, ot * N_TILE : (ot + 1) * N_TILE],
                            in_=y_sb[:, ot * N_TILE : (ot + 1) * N_TILE],
                        )
                    elif ot % 2 == 1:
                        q0 = (ot - 1) * N_TILE
                        nc.sync.dma_start(
                            out=y_d[tb * P : (tb + 1) * P, q0 : q0 + 2 * N_TILE],
                            in_=y_sb[:, q0 : q0 + 2 * N_TILE],
                        )

    _legalize_waits(nc)
    return nc


_CACHED = {}


def kernel(x, u_approx, s_approx, v_approx, _trace=False):
    _install_drain_patch()
    bf16 = ml_dtypes.bfloat16

    vp = (v_approx.astype(np.float32) * s_approx.astype(np.float32)[None, :])
    vc = vp.reshape(I_CHUNKS, P, R)  # [chunk, partition, r]
    vs_host = [
        np.ascontiguousarray(
            vc[VS_STARTS[g] : VS_STARTS[g] + VS_SIZES[g]].transpose(1, 0, 2)
        ).astype(bf16)
        for g in range(VS_GROUPS)
    ]
    ut_host = np.ascontiguousarray(
        np.ascontiguousarray(u_approx.T).reshape(R_HALVES, P, D_OUT)
    ).astype(bf16)
    xT = [np.ascontiguousarray(x[b].T).astype(bf16) for b in range(B)]
    in_maps = [
        {"xT": xT[b], "ut": ut_host}
        | {f"vs{g}": vs_host[g] for g in range(VS_GROUPS)}
        for b in range(B)
    ]

    if "nc" not in _CACHED:
        _CACHED["nc"] = _build()
    res = run_bass_kernel_spmd(_CACHED["nc"], in_maps, list(range(B)), trace=_trace)
    y = np.stack(
        [np.asarray(res.results[b]["y"]).astype(np.float32) for b in range(B)]
    )
    if _trace:
        kernel.last_exec_time_ns = res.exec_time_ns
    return y
